# revision 1
# baseline (speedup 1.0000x reference)
"""V6: baseline per-strip schedule + merged gram/GO MM + Schraudolph exp
+ host-side division. See kernel_baseline.py for the base structure."""

import sys
import numpy as np

sys.path.insert(0, "/opt/trn_rl_repo")

N_CORES = 8
B_TOTAL, M, C, H = 65536, 16, 3, 128
B_CORE = B_TOTAL // N_CORES            # 8192
COLS = B_CORE * M                      # 131072
NB = 512
NPASS = 4
NSTRIP = COLS // NB                    # 256
SUPER = 16                             # strips per output DMA
OUT_W = 16 * SUPER                     # 256 f32 cols per staging tile
CHUNK = 8                              # strips per input DMA

SCL2 = 128.0 * np.log2(np.e)
SC = float(np.sqrt(SCL2))
MAGIC = 16256.0 - 8.0 + 0.5
MASKOFF = 16000.0

_CACHE = {}


def _build(nstrip):
    import concourse.bacc as bacc
    import concourse.tile as tile
    from concourse import mybir

    f32 = mybir.dt.float32
    i16 = mybir.dt.int16
    DT = mybir.dt.bfloat16
    Alu = mybir.AluOpType
    Act = mybir.ActivationFunctionType

    nsuper = max(1, nstrip // SUPER)

    nc = bacc.Bacc("TRN2")
    probT = nc.dram_tensor("probT", [4, COLS], DT, kind="ExternalInput")
    w1 = nc.dram_tensor("w1", [4, H], DT, kind="ExternalInput")
    b1 = nc.dram_tensor("b1", [H, 1], f32, kind="ExternalInput")
    w2z4 = nc.dram_tensor("w2z4", [H, 16], DT, kind="ExternalInput")
    c24 = nc.dram_tensor("c24", [H, 4], f32, kind="ExternalInput")
    bmask = nc.dram_tensor("bmask", [128, 128], f32, kind="ExternalInput")
    m01 = nc.dram_tensor("m01", [128, 128], DT, kind="ExternalInput")
    outb = nc.dram_tensor("outb", [nsuper, 128, OUT_W], f32,
                          kind="ExternalOutput")

    load = {"A": 0.0, "D": 0.0, "G": 0.0}

    def pick(cost_a, cost_d):
        if load["A"] + cost_a <= load["D"] + cost_d:
            load["A"] += cost_a
            return "A"
        load["D"] += cost_d
        return "D"

    with tile.TileContext(nc) as tc:
        from contextlib import ExitStack
        with ExitStack() as ctx:
            singles = ctx.enter_context(tc.tile_pool(name="singles", bufs=1))
            px = ctx.enter_context(tc.tile_pool(name="px", bufs=3))
            pe = ctx.enter_context(tc.tile_pool(name="pe", bufs=3))
            pe0 = ctx.enter_context(tc.tile_pool(name="pe0", bufs=3))
            pem = ctx.enter_context(tc.tile_pool(name="pem", bufs=3))
            pg = ctx.enter_context(tc.tile_pool(name="pg", bufs=4))
            pout = ctx.enter_context(tc.tile_pool(name="pout", bufs=2))
            pH = ctx.enter_context(tc.tile_pool(name="pH", bufs=2, space="PSUM"))
            pS = ctx.enter_context(tc.tile_pool(name="pS", bufs=3, space="PSUM"))

            w1_t = singles.tile([4, H], DT)
            nc.sync.dma_start(out=w1_t, in_=w1[:, :])
            b1_t = singles.tile([H, 1], f32)
            nc.sync.dma_start(out=b1_t, in_=b1[:, :])
            c24_t = singles.tile([H, 4], f32)
            nc.sync.dma_start(out=c24_t, in_=c24[:, :])
            bmask_t = singles.tile([128, 128], f32)
            nc.sync.dma_start(out=bmask_t, in_=bmask[:, :])
            m01_t = singles.tile([128, 128], DT)
            nc.sync.dma_start(out=m01_t, in_=m01[:, :])

            # hT ring: strip tiles [128, 4*132] with w2z at cols 132p+128
            hTs = []
            for i in range(4):
                t = singles.tile([128, 4 * 132], DT, tag=f"hT{i}")
                tv = t[:, :].rearrange("q (g c) -> q g c", g=4)
                nc.sync.dma_start(
                    out=tv[:, :, 128:132],
                    in_=w2z4[:, :].rearrange("q (g c) -> q g c", g=4))
                hTs.append(t)

            outS_box = {}
            chunks = {}

            def st_dma(c):
                xT = px.tile([4, NB * CHUNK], DT, tag="xT")
                nc.sync.dma_start(out=xT, in_=probT[:, NB * CHUNK * c:
                                                    NB * CHUNK * (c + 1)])
                chunks[c] = xT

            def st_hidden(s):
                xT = chunks[s // CHUNK]
                off = (s % CHUNK) * NB
                psumH = pH.tile([128, NB], f32, tag="psumH")
                nc.tensor.matmul(psumH[:, :], w1_t[:, :],
                                 xT[:, off:off + NB], start=True, stop=True)
                hT = hTs[s % 4]
                hv = hT[:, :].rearrange("q (g c) -> q g c", g=4)[:, :, 0:128]
                e = pick(720.0, 721.0)
                if e == "A":
                    nc.scalar.activation(hv, psumH[:, :], Act.Relu,
                                         bias=b1_t[:, 0:1], scale=1.0)
                else:
                    pv = psumH[:, :].rearrange("q (g c) -> q g c", g=4)
                    nc.vector.tensor_scalar(hv, pv, scalar1=b1_t[:, 0:1],
                                            scalar2=0.0, op0=Alu.add,
                                            op1=Alu.max)
                return hT

            def st_gram(s, hT):
                psumS = pS.tile([128, 1024], f32, tag="psumS")
                for p in range(NPASS):
                    base = 132 * p
                    nc.tensor.matmul(psumS[:, 256 * p:256 * p + 132],
                                     hT[:, base:base + 128],
                                     hT[:, base:base + 132],
                                     start=True, stop=True)
                return psumS

            def st_soft(s, psumS):
                sv = psumS[:, :].rearrange("q (g c) -> q g c", g=NPASS)
                cost_d = 686.0
                if max(load["A"], load["D"] + cost_d) <= \
                        max(load["A"] + 720.0, load["D"] + 347.0):
                    load["D"] += cost_d
                    Ei = pe.tile([128, 512], i16, tag="Ei")
                    ev = Ei[:, :].rearrange("q (g c) -> q g c", g=NPASS)
                    nc.vector.tensor_tensor(
                        ev, sv[:, :, 0:128],
                        bmask_t[:, None, :].broadcast_to([128, NPASS, 128]),
                        op=Alu.add)
                    Eb = Ei[:, :].bitcast(mybir.dt.bfloat16)
                else:
                    load["A"] += 720.0
                    load["D"] += 347.0
                    E0 = pe0.tile([128, 512], DT, tag="E0")
                    e0v = E0[:, :].rearrange("q (g c) -> q g c", g=NPASS)
                    nc.scalar.activation(e0v, sv[:, :, 0:128], Act.Exp,
                                         scale=float(1.0 / SCL2))
                    Em = pem.tile([128, 512], DT, tag="Em")
                    emv = Em[:, :].rearrange("q (g c) -> q g c", g=NPASS)
                    nc.vector.tensor_tensor(
                        emv, e0v,
                        m01_t[:, None, :].broadcast_to([128, NPASS, 128]),
                        op=Alu.mult)
                    Eb = Em[:, :]
                # Gpp = GO + c24 (b2 folded; col3 -> ones for the denominator)
                # per-column bias => DVE-only (ACT bias is per-partition)
                gpp = pg.tile([128, 16], DT, tag="Gpp")
                gv = gpp[:, :].rearrange("q (g c) -> q g c", g=NPASS)
                nc.vector.tensor_tensor(
                    gv, sv[:, :, 128:132],
                    c24_t[:, None, :].broadcast_to([128, NPASS, 4]),
                    op=Alu.add)
                load["D"] += 142.0
                return Eb, gpp

            def st_out(s, Eb, gpp, psumS):
                for p in range(NPASS):
                    nc.tensor.matmul(psumS[:, 256 * p + 136:256 * p + 140],
                                     Eb[:, 128 * p:128 * p + 128],
                                     gpp[:, 4 * p:4 * p + 4],
                                     start=True, stop=True)

            def st_ocopy(s, psumS):
                su, t = divmod(s, SUPER)
                if t == 0:
                    outS = pout.tile([128, OUT_W], f32, tag="outS")
                    outS_box["t"] = outS
                outS = outS_box["t"]
                sv = psumS[:, :].rearrange("q (g c) -> q g c", g=NPASS)
                ov = outS[:, 16 * t:16 * (t + 1)].rearrange(
                    "q (g c) -> q g c", g=NPASS)
                e = pick(307.0, 142.0)
                if e == "A":
                    nc.scalar.copy(ov, sv[:, :, 136:140])
                else:
                    nc.vector.tensor_copy(ov, sv[:, :, 136:140])
                if t == SUPER - 1 or s == nstrip - 1:
                    nc.sync.dma_start(out=outb[su, :, :], in_=outS[:, :])

            # baseline-style software pipeline, per strip, depth 2
            live = {}
            st_dma(0)
            live[0] = [st_hidden(0)]
            live[1] = [st_hidden(1)]
            for i in range(nstrip + 2):
                if i % CHUNK == 0 and (i + CHUNK) // CHUNK < (nstrip + CHUNK - 1) // CHUNK:
                    st_dma((i + CHUNK) // CHUNK)
                if i + 2 < nstrip:
                    live[i + 2] = [st_hidden(i + 2)]
                if i - 2 >= 0:
                    s4 = live.pop(i - 2)
                    st_out(i - 2, s4[2], s4[3], s4[1])
                    st_ocopy(i - 2, s4[1])
                if 0 <= i - 1 < nstrip:
                    s2 = live[i - 1]
                    psumS = st_gram(i - 1, s2[0])
                    s2.append(psumS)
                    Eb, gpp = st_soft(i - 1, psumS)
                    s2.append(Eb)
                    s2.append(gpp)

    nc.finalize()
    return nc


def _prep_core_inputs(prob_core, W1, b1, W2, b2):
    import ml_dtypes
    bf16 = ml_dtypes.bfloat16
    pT = np.ascontiguousarray(prob_core.reshape(-1, C).T)
    idx = np.tile(np.arange(M, dtype=np.float32), B_CORE)[None]
    probT_aug = np.ascontiguousarray(np.concatenate([pT, idx], axis=0))

    W1s = np.asarray(W1, np.float32) * SC
    b1s = np.asarray(b1, np.float32).reshape(H, 1) * SC
    w2s = np.asarray(W2, np.float32) / SC
    w2z = np.concatenate([w2s, np.zeros((H, 1), np.float32)], axis=1)
    w2z4 = np.tile(w2z, (1, 4))
    c24 = np.concatenate([np.broadcast_to(
        np.asarray(b2, np.float32)[None, :], (H, C)),
        np.ones((H, 1), np.float32)], axis=1)

    mask = np.kron(np.eye(8, dtype=np.float32), np.ones((16, 16), np.float32))
    bmask = (MAGIC - MASKOFF * (1.0 - mask)).astype(np.float32)

    return {
        "probT": probT_aug.astype(bf16),
        "w1": np.ascontiguousarray(W1s).astype(bf16),
        "b1": np.ascontiguousarray(b1s),
        "w2z4": np.ascontiguousarray(w2z4).astype(bf16),
        "c24": np.ascontiguousarray(c24),
        "bmask": np.ascontiguousarray(bmask),
        "m01": np.ascontiguousarray(mask).astype(bf16),
    }


def _postprocess(outb_arr):
    # outb [nsuper, 128, 256]: [su, q=(e,m), 16*t + 4*p + cc]
    # strip s = su*16 + t ; b = 32*s + 8*p + e
    nsuper = outb_arr.shape[0]
    r = outb_arr.reshape(nsuper, 8, 16, SUPER, NPASS, 4)  # su, e, m, t, p, cc
    r = r.transpose(0, 3, 4, 1, 2, 5)                     # su, t, p, e, m, cc
    r = r.reshape(-1, M, 4)
    return np.ascontiguousarray(r[..., 0:3] / r[..., 3:4])


def kernel(prob, W1, b1, W2, b2, _trace=False):
    from concourse.bass_utils import run_bass_kernel_spmd

    if "nc" not in _CACHE:
        _CACHE["nc"] = _build(NSTRIP)
    nc = _CACHE["nc"]

    prob = np.asarray(prob, np.float32)
    in_maps = []
    for ci in range(N_CORES):
        pc = prob[ci * B_CORE:(ci + 1) * B_CORE]
        in_maps.append(_prep_core_inputs(pc, W1, b1, W2, b2))
    res = run_bass_kernel_spmd(nc, in_maps, list(range(N_CORES)),
                               trace=_trace)
    _CACHE["last_result"] = res
    out = np.zeros((B_TOTAL, M, C), np.float32)
    for ci in range(N_CORES):
        o = _postprocess(res.results[ci]["outb"])
        out[ci * B_CORE:ci * B_CORE + o.shape[0]] = o
    return out

